# revision 55
# baseline (speedup 1.0000x reference)
"""Trainium2 Bass kernel for nn_AutoIntTPPSameInfluence — exp-sum formulation.

dF(x) (the scalar derivative of the 1->64->64->64->1 tanh MLP) is fit on host
as sums of decaying exponentials, split by range:
  near (x < X0):   8 rates, 16 points per segment column (8*16 = 128)
  far (X0<=x<W):   4 rates, 32 points per segment column (4*32 = 128)
  x >= W:          dropped (dF tail integral < 1e-4 -> per-lambda error ~3e-4)
On device every pairwise influence point is ONE table-exp evaluation: points
are packed SEG-per-column and replicated over the partition groups of a
[128, N] tile, a single ACT instruction computes |c_i| exp(-s_i x + ln|c_i|)
for all rates via per-partition scale/bias, and one bf16 matmul with a +-1
selector contracts all 128 partitions — performing the SEG-point segment sum
AND the weighted rate sum at 1 col/cycle.  Masked/padded points use x = 3e4,
driving every exponential to exactly 0.

The integral term sum_k F(T_END - t_k) - F0 only needs a bulk sum, so
F(x) - F0 is fit the same way (const + 8 exps) and rides through the same
pipeline as the leading FT columns; the constant is applied on host via the
valid count.  Host: scatter segment sums to events, log/mask/reduce in f64.

Schedule notes: a dummy ACT on a framework const AP forces the exp
ACT_TABLE_LOAD to run during the input-DMA completion window (~2.5 us fixed
DMA latency); const DMAs go first on the sync queue and are consumed without
staging copies; F columns lead so their chain never sits in the tail; PSUM
accumulation groups let output DMAs start mid-run.
"""

import numpy as np
from contextlib import ExitStack

import ml_dtypes

import concourse.bacc as bacc
import concourse.tile as tile
import concourse.mybir as mybir
from concourse.bass_utils import run_bass_kernel_spmd

B, L, H = 16, 320, 64
T_END = 100.0
NC = 8
SEGN = 16                   # F: points per column, 8 rates
SEGN2 = 8                   # near: 8 points x 8 rates x 2 events per column
SEGR = 32                   # far: 32 points x 2 rates x 2 events per column
X0 = 4.0                    # near/far boundary
WCUT = 12.0                 # truncation window
GS = 4                      # tiles per PSUM accumulation group
TS_MAX = 512                # PSUM bank width in f32
XPAD = np.float32(30000.0)  # pad x: exp(-s*XPAD) underflows to exactly 0
BF16 = mybir.dt.bfloat16
F32 = mybir.dt.float32
Exp = mybir.ActivationFunctionType.Exp
NPBF16 = ml_dtypes.bfloat16


# ---------------------------------------------------------------- host fits
_FIT_CACHE = {}


def _mlp_funcs(W1, b1, W2, b2, W3, b3, W4, b4):
    w1 = W1[:, 0].astype(np.float64)
    b1d, b2d, b3d = (b1.astype(np.float64), b2.astype(np.float64),
                     b3.astype(np.float64))
    W2d, W3d, W4d = (W2.astype(np.float64), W3.astype(np.float64),
                     W4.astype(np.float64))
    b4d = float(np.asarray(b4, np.float64)[0])

    def dF(x):
        x = np.ravel(x)
        h1 = np.outer(w1, x) + b1d[:, None]
        a1 = np.tanh(h1)
        d1 = (1 - a1 ** 2) * w1[:, None]
        h2 = W2d @ a1 + b2d[:, None]
        a2 = np.tanh(h2)
        d2 = (1 - a2 ** 2) * (W2d @ d1)
        h3 = W3d @ a2 + b3d[:, None]
        a3 = np.tanh(h3)
        d3 = (1 - a3 ** 2) * (W3d @ d2)
        return (W4d @ d3)[0]

    def F(x):
        x = np.ravel(x)
        h1 = np.tanh(np.outer(w1, x) + b1d[:, None])
        h2 = np.tanh(W2d @ h1 + b2d[:, None])
        h3 = np.tanh(W3d @ h2 + b3d[:, None])
        return (W4d @ h3)[0] + b4d

    return dF, F


def _ridge_fit(A, y, lam):
    cn = np.linalg.norm(A, axis=0)
    return np.linalg.solve(A.T @ A + lam * np.diag(cn ** 2), A.T @ y)


def _scan_fit(gx, gy, floor, nk, smins, smaxs, cmax):
    wts = 1.0 / (np.abs(gy) + floor)
    best = None
    for smin in smins:
        for smax in smaxs:
            r = np.geomspace(smin, smax, nk)
            A = np.exp(-np.outer(gx, r)) * wts[:, None]
            c = _ridge_fit(A, gy * wts, 1e-5)
            if np.abs(c).max() > cmax:
                continue
            werr = np.abs((np.exp(-np.outer(gx, r)) @ c - gy) * wts).max()
            if best is None or werr < best[0]:
                best = (werr, r, c)
    return best[1], best[2]


def _fits(W1, b1, W2, b2, W3, b3, W4, b4):
    key = b"".join(np.ascontiguousarray(a).tobytes()
                   for a in (W1, b1, W2, b2, W3, b3, W4, b4))
    if key in _FIT_CACHE:
        return _FIT_CACHE[key]
    dF, F = _mlp_funcs(W1, b1, W2, b2, W3, b3, W4, b4)
    F0 = float(F(np.zeros(1))[0])

    gx = np.unique(np.concatenate([np.geomspace(1e-4, X0 * 1.02, 4000),
                                   np.linspace(0, X0 * 1.02, 4000)]))
    rn, cn = _scan_fit(gx, dF(gx), 1e-4, 8,
                       (0.3, 0.5, 0.8, 1.2), (10., 14., 18., 24., 30.), 300.)
    gxf = np.linspace(X0 * 0.98, WCUT * 1.02, 6000)
    rf, cf = _scan_fit(gxf, dF(gxf), 2e-6, 2,
                       (0.1, 0.15, 0.2, 0.3, 0.4),
                       (0.5, 0.7, 0.9, 1.2, 1.8), 1e3)

    gxF = np.linspace(0, 100, 20001)
    gyF = F(gxF) - F0
    rF = np.geomspace(0.04, 16.0, 8)
    AF = np.concatenate([np.ones((len(gxF), 1)),
                         np.exp(-np.outer(gxF, rF))], axis=1)
    cfF = _ridge_fit(AF, gyF, 1e-7)
    CF, cF = float(cfF[0]), cfF[1:]

    out = dict(rn=rn, cn=cn, rf=rf, cf=cf, rF=rF, cF=cF, CF=CF, F0=F0)
    _FIT_CACHE[key] = out
    return out


# ---------------------------------------------------------------- packing
def _seg_stream(t, lens, lo_f, hi_f, seg):
    """Segments of `seg` consecutive j's with t_k - t_j in [lo, hi) per event.
    Returns x [Tseg, seg] f32 (XPAD-padded), ev [Tseg] (b*L + k)."""
    xs, evs = [], []
    for b in range(B):
        n = int(lens[b])
        tb = t[b, :n].astype(np.float64)
        jhi = np.searchsorted(tb, tb - lo_f) if lo_f > 0 else np.arange(n)
        jlo = np.searchsorted(tb, tb - hi_f) if hi_f is not None else \
            np.zeros(n, np.int64)
        jhi = np.minimum(jhi, np.arange(n))
        cnt = jhi - jlo
        nsg = (cnt + seg - 1) // seg
        ev_idx = np.repeat(np.arange(n), nsg)
        starts = np.concatenate([[0], np.cumsum(nsg)[:-1]])
        within = (np.arange(int(nsg.sum())) - np.repeat(starts, nsg)) * seg
        j0 = jlo[ev_idx] + within
        jj = j0[:, None] + np.arange(seg)[None, :]
        valid = jj < jhi[ev_idx][:, None]
        jc = np.minimum(jj, n - 1)
        x = np.where(valid, tb[ev_idx][:, None] * 0 + (t[b, ev_idx][:, None]
                     - tb[jc]), XPAD).astype(np.float32)
        xs.append(x)
        evs.append(b * L + ev_idx)
    return np.concatenate(xs), np.concatenate(evs)


def _layout_stream(x, ev, seg, nrep):
    """Split stream across cores; -> xcore [NC, 128, cols_pc], ev [NC, cols],
    (NT, SEGT)."""
    Tseg = x.shape[0]
    Spc = (Tseg + NC - 1) // NC
    NT = (Spc + TS_MAX - 1) // TS_MAX
    SEGT = ((Spc + NT - 1) // NT + 7) // 8 * 8
    cap = NC * NT * SEGT
    xp = np.full((cap, seg), XPAD, np.float32)
    xp[:Tseg] = x
    evp = np.full(cap, -1, np.int64)
    evp[:Tseg] = ev
    xc = xp.reshape(NC, NT * SEGT, seg).transpose(0, 2, 1)  # [NC, seg, cols]
    xc = np.tile(xc, (1, nrep, 1))                          # [NC, 128, cols]
    return xc, evp.reshape(NC, NT * SEGT), NT, SEGT


def _layout_paired(x, ev, seg):
    """Pair two seg-point segments per column: partitions 0-63 carry segment
    A replicated over 64//seg rate groups, 64-127 segment B.  -> xcore
    [NC, 128, cols], evA/evB [NC, cols], (NT, SEGT)."""
    Tseg = x.shape[0]
    npair = (Tseg + 1) // 2
    Spc = (npair + NC - 1) // NC
    NT = (Spc + TS_MAX - 1) // TS_MAX
    SEGT = ((Spc + NT - 1) // NT + 7) // 8 * 8
    cap = NC * NT * SEGT
    xp = np.full((cap * 2, seg), XPAD, np.float32)
    xp[:Tseg] = x
    evp = np.full(cap * 2, -1, np.int64)
    evp[:Tseg] = ev
    xp = xp.reshape(NC, NT * SEGT, 2, seg)
    A, B = xp[:, :, 0, :], xp[:, :, 1, :]                   # [NC, cols, seg]
    nrep = 64 // seg
    xc = np.concatenate([A] * nrep + [B] * nrep, axis=2).transpose(0, 2, 1)
    evp = evp.reshape(NC, NT * SEGT, 2)
    return xc, evp[:, :, 0], evp[:, :, 1], NT, SEGT


def _pack(t, lens):
    xn, evn = _seg_stream(t, lens, 0.0, X0, SEGN2)
    xr, evr = _seg_stream(t, lens, X0, WCUT, SEGR)
    xcn, evnA, evnB, TN, SEGTN = _layout_paired(xn, evn, SEGN2)
    xcr, evrA, evrB, TR, SEGTR = _layout_paired(xr, evr, SEGR)

    # F points: one per valid event (all k < n)
    fx = []
    for b in range(B):
        n = int(lens[b])
        fx.append((T_END - t[b, :n]).astype(np.float32))
    fx = np.concatenate(fx)
    nF = len(fx)
    nFc = (nF + NC - 1) // NC
    FT = ((nFc + SEGN - 1) // SEGN + 1) // 2 * 2
    xf = np.full((NC * FT * SEGN,), XPAD, np.float32)
    xf[:nF] = fx
    xf = xf.reshape(NC, FT, SEGN).transpose(0, 2, 1)
    xf = np.tile(xf, (1, 128 // SEGN, 1))                   # [NC, 128, FT]

    assert SEGTN + FT <= TS_MAX
    xbb = np.concatenate([xf, xcr, xcn], axis=2)            # [NC, 128, XC]
    return (xbb, (evnA, evnB, evrA, evrB),
            (TN, SEGTN, TR, SEGTR, FT), nF)


# ---------------------------------------------------------------- program
_PROGRAM_CACHE = {}


def build_program(TN, SEGTN, TR, SEGTR, FT):
    pkey = (TN, SEGTN, TR, SEGTR, FT)
    if pkey in _PROGRAM_CACHE:
        return _PROGRAM_CACHE[pkey]
    XC = FT + TN * SEGTN + TR * SEGTR
    NGn = (TN + GS - 1) // GS
    NGr = (TR + GS - 1) // GS
    OUTW = NGr * SEGTR + NGn * SEGTN + FT    # single staged output row
    NB = 2 * GS * GS                         # near selector block width
    SELW = 2 * NB + 2 * GS                   # near | far (2 rows/tile) | F
    nc = bacc.Bacc("TRN2", target_bir_lowering=False, debug=False,
                   enable_asserts=False)

    xbb_d = nc.dram_tensor("xbb", [128, XC], BF16, kind="ExternalInput")
    selv_d = nc.dram_tensor("selv", [128, SELW], BF16, kind="ExternalInput")
    cf_d = nc.dram_tensor("cfd", [128, 6], F32, kind="ExternalInput")
    outs_d = nc.dram_tensor("out_s", [2 * GS, OUTW], F32,
                            kind="ExternalOutput")

    # tiles: ("R", i) far first, ("N", i) near last; chunks pair tiles
    # within a stream; the F columns get their own leading chunk
    tiles = [("R", i) for i in range(TR)] + [("N", i) for i in range(TN)]
    chunks = []
    i = 0
    while i < len(tiles):
        if i + 1 < len(tiles) and tiles[i][0] == tiles[i + 1][0]:
            chunks.append(tiles[i:i + 2])
            i += 2
        else:
            chunks.append(tiles[i:i + 1])
            i += 1

    def col0(tl):
        s, i = tl
        return FT + TR * SEGTR + i * SEGTN if s == "N" else FT + i * SEGTR

    def width(tl):
        return SEGTN if tl[0] == "N" else SEGTR

    with tile.TileContext(nc) as tc, ExitStack() as ctx, \
            nc.allow_low_precision(reason="bf16 exp terms; tol is 2e-2"):
        consts = ctx.enter_context(tc.tile_pool(name="consts", bufs=1))
        xb_p = ctx.enter_context(tc.tile_pool(name="xb", bufs=len(chunks)))
        term_p = ctx.enter_context(tc.tile_pool(name="term", bufs=3))
        outp_p = ctx.enter_context(tc.tile_pool(name="outp", bufs=4,
                                                space="PSUM"))
        stage_p = ctx.enter_context(tc.tile_pool(name="stage", bufs=2))

        # const DMAs ride the scalar queue ahead of the dummy ACT, so the
        # compiler-inserted ACT_TABLE_LOAD for Exp (bound to the dummy, which
        # has zero data deps) overlaps their ~2.5 us completion latency
        cfc = consts.tile([128, 6], F32, tag="cfc")
        nc.scalar.dma_start(out=cfc[:], in_=cf_d.ap(), single_packet=True)
        selc = consts.tile([128, SELW], BF16, tag="selc")
        nc.scalar.dma_start(out=selc[:], in_=selv_d.ap())
        zeros_ap = nc.const_aps.aps[(mybir.dt.float32, 0.0)]
        dummy = consts.tile([128, 1], F32, tag="dummy")
        nc.scalar.activation(dummy[:], zeros_ap, Exp)

        # chunk 0 carries the FT leading F columns along with the first far
        # tiles so every ACT's data rides the earliest DMA completions
        xbts = []
        for ci, ch in enumerate(chunks):
            a = 0 if ci == 0 else col0(ch[0])
            z = col0(ch[-1]) + width(ch[-1])
            xbt = xb_p.tile([128, z - a], BF16, tag="xb", name=f"xb{ci}")
            nc.sync.dma_start(out=xbt[:], in_=xbb_d.ap()[:, a:z])
            xbts.append((z - a - (FT if ci == 0 else 0), xbt))

        # F chain first: its ACT/MM/copy/DMA never sit in the tail
        termf = term_p.tile([128, FT], BF16, tag="termf")
        nc.scalar.activation(termf[:], xbts[0][1][:, :FT], Exp,
                             bias=cfc[:, 3:4], scale=cfc[:, 2:3])

        cpair = {"N": (cfc[:, 0:1], cfc[:, 1:2]),
                 "R": (cfc[:, 5:6], cfc[:, 4:5])}
        gF = NGr                                   # F rides near group 0

        def gcol(g):                               # group -> stage col offset
            if g < NGr:
                return g * SEGTR
            return NGr * SEGTR + (SEGTN + FT if g > gF else 0) \
                + (g - NGr - 1 if g > gF else 0) * SEGTN

        def gwidth(g):
            if g < NGr:
                return SEGTR
            return SEGTN + FT if g == gF else SEGTN

        stage = stage_p.tile([2 * GS, OUTW], F32, tag="stage")
        outbs = {}
        for ci, ch in enumerate(chunks):
            cw, xbt = xbts[ci]
            a = FT if ci == 0 else 0
            tt = term_p.tile([128, cw], BF16, tag="terms",
                             name=f"terms{ci}")
            sc, bi = cpair[ch[0][0]]
            nc.scalar.activation(tt[:], xbt[:, a:], Exp, bias=bi, scale=sc)
            for j, tl in enumerate(ch):
                s, i = tl
                w = width(tl)
                g = i // GS + (NGr if s == "N" else 0)
                tg = i % GS
                in_g = min(GS, (TN if s == "N" else TR) - GS * (i // GS))
                if tg == 0:
                    outbs[g] = outp_p.tile([2 * GS, gwidth(g)], F32,
                                           tag="outb", name=f"outb{g}")
                    if g == gF:
                        nc.tensor.matmul(out=outbs[g][:, SEGTN:],
                                         lhsT=selc[:, 2 * NB:],
                                         rhs=termf[:], start=True, stop=True)
                off = 0 if s == "N" else NB
                lh = selc[:, off + 2 * GS * tg:off + 2 * GS * (tg + 1)]
                nc.tensor.matmul(out=outbs[g][:, :w], lhsT=lh,
                                 rhs=tt[:, j * w:(j + 1) * w],
                                 start=(tg == 0), stop=(tg == in_g - 1))
                if tg == in_g - 1:
                    nc.vector.tensor_copy(
                        stage[:, gcol(g):gcol(g) + gwidth(g)],
                        outbs[g][:])
        nc.sync.dma_start(out=outs_d.ap(), in_=stage[:])

    nc.compile()
    prog = (nc, pkey)
    _PROGRAM_CACHE[pkey] = prog
    return prog


# ---------------------------------------------------------------- kernel
def _prepare(seq_pads, background, W1, b1, W2, b2, W3, b3, W4, b4, seq_lens):
    t = np.asarray(seq_pads)[:, :, 0].astype(np.float32)
    lens = np.asarray(seq_lens).astype(np.int64)
    ft = _fits(
        np.asarray(W1, np.float64), np.asarray(b1, np.float64),
        np.asarray(W2, np.float64), np.asarray(b2, np.float64),
        np.asarray(W3, np.float64), np.asarray(b3, np.float64),
        np.asarray(W4, np.float64), np.asarray(b4, np.float64))

    xbb, (evnA, evnB, evrA, evrB), dims, nF = _pack(t, lens)
    TN, SEGTN, TR, SEGTR, FT = dims
    nc, _ = build_program(*dims)

    g16 = np.repeat(np.arange(8), SEGN)        # F: partition -> rate
    g8p = np.tile(np.repeat(np.arange(8), SEGN2), 2)   # near paired
    g32p = np.tile(np.repeat(np.arange(2), SEGR), 2)   # far paired
    cfd = np.zeros((128, 6), np.float32)
    cfd[:, 0] = -ft["rn"][g8p]
    cfd[:, 1] = np.log(np.maximum(np.abs(ft["cn"]), 1e-20))[g8p]
    cfd[:, 2] = -ft["rF"][g16]
    cfd[:, 3] = np.log(np.maximum(np.abs(ft["cF"]), 1e-20))[g16]
    cfd[:, 5] = -ft["rf"][g32p]
    cfd[:, 4] = np.log(np.maximum(np.abs(ft["cf"]), 1e-20))[g32p]
    NB = 2 * GS * GS
    SELW = 2 * NB + 2 * GS
    selv = np.zeros((128, SELW), np.float32)
    sgn_n = np.sign(ft["cn"])[g8p]
    sgn_r = np.sign(ft["cf"])[g32p]
    half = np.arange(128) >= 64                # B-segment partitions
    for v in range(GS):
        selv[~half, 2 * GS * v + v] = sgn_n[~half]
        selv[half, 2 * GS * v + GS + v] = sgn_n[half]
        selv[~half, NB + 2 * GS * v + v] = sgn_r[~half]
        selv[half, NB + 2 * GS * v + GS + v] = sgn_r[half]
    selv[:, 2 * NB] = np.sign(ft["cF"])[g16]
    selv = selv.astype(NPBF16)

    in_maps = []
    for cix in range(NC):
        m = dict(selv=selv, cfd=cfd)
        m["xbb"] = np.ascontiguousarray(xbb[cix].astype(NPBF16))
        in_maps.append(m)
    meta = dict(evnA=evnA, evnB=evnB, evrA=evrA, evrB=evrB, dims=dims,
                nF=nF, CF=ft["CF"], lens=lens)
    return nc, in_maps, meta


def kernel(seq_pads, background, W1, b1, W2, b2, W3, b3, W4, b4, seq_lens):
    nc, in_maps, meta = _prepare(seq_pads, background, W1, b1, W2, b2, W3,
                                 b3, W4, b4, seq_lens)
    TN, SEGTN, TR, SEGTR, FT = meta["dims"]
    NGn = (TN + GS - 1) // GS
    NGr = (TR + GS - 1) // GS
    lens = meta["lens"]

    def decode(res):
        """-> (nearA, nearB, farA, farB) partials, f_exp."""
        pna, pnb, pa, pb, fe = [], [], [], [], 0.0
        for cc in range(NC):
            o = res[cc]["out_s"]                   # [2*GS, OUTW]
            for g in range(NGr):
                in_g = min(GS, TR - GS * g)
                cols = slice(g * SEGTR, (g + 1) * SEGTR)
                pa.append(o[:in_g, cols].reshape(-1))
                pb.append(o[GS:GS + in_g, cols].reshape(-1))
            base = NGr * SEGTR
            for g in range(NGn):
                c0 = base + (SEGTN + FT) * min(g, 1) + max(g - 1, 0) * SEGTN
                in_g = min(GS, TN - GS * g)
                pna.append(o[:in_g, c0:c0 + SEGTN].reshape(-1))
                pnb.append(o[GS:GS + in_g, c0:c0 + SEGTN].reshape(-1))
            fe += o[:, base + SEGTN:base + SEGTN + FT].sum()
        return (np.concatenate(pna), np.concatenate(pnb),
                np.concatenate(pa), np.concatenate(pb), fe)

    res = run_bass_kernel_spmd(nc, in_maps, core_ids=list(range(NC))).results
    parts = decode(res)
    if not all(np.isfinite(p).all() for p in parts[:4]) \
            or not np.isfinite(parts[4]):
        res = run_bass_kernel_spmd(nc, in_maps,
                                   core_ids=list(range(NC))).results
        parts = decode(res)
    pna, pnb, pa, pb, f_exp = parts

    S = np.zeros(B * L, np.float64)
    for ev, p in ((meta["evnA"].reshape(-1), pna),
                  (meta["evnB"].reshape(-1), pnb),
                  (meta["evrA"].reshape(-1), pa),
                  (meta["evrB"].reshape(-1), pb)):
        ok = ev >= 0
        np.add.at(S, ev[ok], p[ok].astype(np.float64))
    S = S.reshape(B, L)

    bg = float(np.asarray(background)[0])
    lam = np.maximum(bg + S, 1e-12)        # fit wiggle must never reach log(<=0)
    mask = np.arange(L)[None, :] < lens[:, None]
    sum_log = np.log(np.where(mask, lam, 1.0)).sum()

    ints_total = f_exp + meta["nF"] * meta["CF"] + B * T_END * bg
    nll = -(sum_log - ints_total) / B
    return np.float32(nll)
